# revision 4
# baseline (speedup 1.0000x reference)
"""Trainium2 Bass kernel for the ragged-sequence attention problem.

Math (per batch row):
    u      = tanh(h @ W.T + b)          h: [L, H]
    s      = u @ u_s                    masked to length, then softmax
    v      = sum_l alpha_l * h_l

Strategy (v5 — v4 + fused exp pairs + TensorTensorReduce for v):
  - Length-aware schedule: rows are LPT-packed onto 8 cores (32 rows each,
    balanced by tile count), each core's rows sorted by T descending.  The
    SPMD program processes slot i with T_seq[i] = max over cores of that
    rank's T; padding overhead is ~2% of tiles.
  - h is pre-cast to bf16, padded to 256 channels, and pre-transposed on
    the host into the exact SBUF layout; channel 255 is ONES (carries the
    bias through the u-matmul and makes the softmax denominator fall out
    of the v-reduction for free).  Rows at l >= len are zero everywhere,
    so masking costs nothing on device.
  - u-matmul: 4 matmuls per 512-l group (2 k-chunks x 2 m-chunks), one
    bias-free tanh ACT over both m-chunks of a 2-bank PSUM tile.
  - scores: u_s replicated to 128 stationary columns on the host, so the
    scores matmul emits 128 IDENTICAL score rows into PSUM; the Exp ACT
    then yields the alpha BROADCAST in bf16 directly.  Scores of TWO
    consecutive 512-l groups land in one 2-bank PSUM pair tile and are
    exponentiated by ONE ACT (halves the exp instruction count; the ACT
    engine is the bottleneck at ~96us theoretical).
  - v: per pair and per 128-channel chunk, ONE DVE tensor_tensor_reduce
    computes sum_l alpha_l*h[c,l] directly into the resident vfin column
    (accum chained across pairs via the scalar init operand); the full
    product tensor is discarded through a stride-0 dummy output.  This
    replaces v4's tensor_mul + tensor_reduce + per-row fold (4N+ DVE
    cycles/group at 1-2x) with ~N cycles/group at 1x and 4x fewer DVE
    instructions (fewer DRAIN stalls).
  - Software pipeline over (row, group) units: scores lag u by 1 unit,
    exp fires 2 units after a pair's last group, the v-TTRs 3 units after.
"""

import sys

import numpy as np

sys.path.insert(0, "/opt/trn_rl_repo")

import ml_dtypes  # noqa: E402

import concourse.bass as bass  # noqa: E402
import concourse.mybir as mybir  # noqa: E402
import concourse.tile as tile  # noqa: E402
from concourse.bass_utils import run_bass_kernel_spmd  # noqa: E402
import bass_rust as _br  # noqa: E402

N_CORES = 8
B, L, H = 256, 2048, 240
BPC = B // N_CORES        # batch rows per core
HP = 256                  # h channels padded (two 128 chunks)
H1 = H - 128              # 112 real channels in chunk 1
GSZ = 512
F32 = mybir.dt.float32
BF16 = mybir.dt.bfloat16
AF = mybir.ActivationFunctionType
ALU = mybir.AluOpType
AX = mybir.AxisListType
BF16NP = ml_dtypes.bfloat16

_MAXW = 1  # sync waits kept on an instruction; the rest move to nops


class _TC(tile.TileContext):
    """Walrus in this container caps sync-wait commands per instruction
    ("Too many sync wait commands"), but Tile freely attaches one wait per
    producer semaphore.  After scheduling, hoist excess waits onto dedicated
    single-wait nops inserted just before the instruction on its engine."""

    def schedule_and_allocate(self, validate_deps=False):
        ret = super().schedule_and_allocate(validate_deps)
        self._split_excess_waits()
        return ret

    def _split_excess_waits(self):
        nc = self.nc
        n_split = 0
        for fn in nc.m.functions:
            for bb in fn.blocks:
                insts = bb.instructions
                i = 0
                while i < len(insts):
                    inst = insts[i]
                    si = getattr(inst, "sync_info", None)
                    waits = list(si.on_wait) if si is not None else []
                    if len(waits) > _MAXW:
                        si.on_wait = waits[-_MAXW:]
                        inst.sync_info = si
                        for w in waits[:-_MAXW]:
                            nop = mybir.InstNoOp(
                                name=f"waitsplit-{n_split}", ins=[], outs=[])
                            n_split += 1
                            nop.engine = inst.engine
                            nop.sync_info = _br.SyncInfo(
                                on_wait=[w], on_update=[])
                            nc.register_instruction(nop, overwrite=True)
                            insts.insert(i, nop)
                            i += 1
                    i += 1


def _schedule(lens):
    """Snake-pack rows onto cores: global sort by tile count descending,
    assign ranks in snake order (0..7, 7..0, ...).  Rank i's cross-core max
    is then the (8i)-th order statistic, so the shared T_seq wastes only
    ~3% of tiles (vs ~10% for load-only LPT), while per-core loads stay
    balanced.  Returns (perm [8][32] row ids, T_seq [32])."""
    lens = np.asarray(lens).astype(np.int64)
    T = np.ceil(lens / 16).astype(np.int64)   # 16-l sub-tiles
    order = np.argsort(-T, kind="stable")
    perm = [[] for _ in range(N_CORES)]
    for i in range(BPC):
        blk = order[N_CORES * i:N_CORES * (i + 1)]
        if i % 2 == 1:
            blk = blk[::-1]
        for c in range(N_CORES):
            perm[c].append(int(blk[c]))
    T_seq = [max(int(T[perm[c][i]]) for c in range(N_CORES))
             for i in range(BPC)]
    return perm, tuple(T_seq)


def build(T_seq):
    nc = bass.Bass("TRN2", target_bir_lowering=False, debug=False,
                   num_devices=N_CORES)
    offs = []
    tot = 0
    for t in T_seq:
        offs.append(tot)
        tot += 2 * 16 * t
    h_d = nc.declare_dram_parameter("hT", [128, tot], BF16, isOutput=False)
    w0_d = nc.declare_dram_parameter("wtb0", [128, HP], BF16, isOutput=False)
    w1_d = nc.declare_dram_parameter("wtb1", [128, HP], BF16, isOutput=False)
    u0_d = nc.declare_dram_parameter("usr0", [128, BPC * 128], BF16,
                                     isOutput=False)
    u1_d = nc.declare_dram_parameter("usr1", [128, BPC * 128], BF16,
                                     isOutput=False)
    h0_d = nc.declare_dram_parameter("uh0", [128, 8 * 128], BF16,
                                     isOutput=False)
    h1_d = nc.declare_dram_parameter("uh1", [128, 8 * 128], BF16,
                                     isOutput=False)
    ov_d = nc.declare_dram_parameter("ov", [128, 2 * BPC], F32, isOutput=True)

    with _TC(nc) as tc:
        with (
            tc.tile_pool(name="consts", bufs=1) as cp,
            tc.tile_pool(name="ht", bufs=1) as htp,
            tc.tile_pool(name="ut", bufs=4) as utp,
            tc.tile_pool(name="ab", bufs=3) as abp,
            tc.tile_pool(name="dm", bufs=4) as dmp,
            tc.tile_pool(name="pu", bufs=2, space="PSUM") as pup,
            tc.tile_pool(name="sg", bufs=2, space="PSUM") as sgp,
        ):
            wtb0 = cp.tile([128, HP], BF16)
            wtb1 = cp.tile([128, HP], BF16)
            usr0 = cp.tile([128, BPC * 128], BF16)
            usr1 = cp.tile([128, BPC * 128], BF16)
            vfin = cp.tile([128, 2 * BPC], F32)
            uh0 = cp.tile([128, 8 * 128], BF16)
            uh1 = cp.tile([128, 8 * 128], BF16)
            # weights + an 8-slot u_s head go on the fast HWDGE ring ahead
            # of the h stream (the gpsimd SWDGE delivers the big replicated
            # u_s tables only by ~20-30us, far too late for the first rows)
            nc.sync.dma_start(wtb0[:], w0_d.ap()[:, :])
            nc.sync.dma_start(wtb1[:], w1_d.ap()[:, :])
            nc.sync.dma_start(uh0[:], h0_d.ap()[:, :])
            nc.sync.dma_start(uh1[:], h1_d.ap()[:, :])
            nc.gpsimd.dma_start(usr0[:], u0_d.ap()[:, :])
            nc.gpsimd.dma_start(usr1[:], u1_d.ap()[:, :])

            class Row:
                pass

            def new_row(i, T):
                # exact-sized per-row tile; ALL rows resident at once.
                # h arrives HOST-PRE-TRANSPOSED in the exact SBUF layout,
                # so the load is one plain contiguous DMA (~358 GB/s).
                r = Row()
                r.i, r.T = i, T
                r.G = (16 * T + GSZ - 1) // GSZ
                r.L2 = 16 * T
                W = 2 * 16 * T
                r.ht = htp.tile([128, W], BF16, tag=f"ht{i}")
                nc.sync.dma_start(r.ht[:],
                                  h_d.ap()[:, offs[i]:offs[i] + W])
                r.ut, r.sg, r.ab = {}, {}, {}
                return r

            def nsz(r, g):
                return min(GSZ, 16 * r.T - g * GSZ)

            def pair_span(r, p):
                # l-span of pair p: groups 2p and (2p+1 if present).
                # Non-final groups are always full, so the span is
                # contiguous starting at l = 2p*GSZ.
                n = nsz(r, 2 * p)
                if 2 * p + 1 < r.G:
                    n += nsz(r, 2 * p + 1)
                return n

            def emit_u(r, g):
                N = nsz(r, g)
                gs = slice(g * GSZ, g * GSZ + N)
                gs1 = slice(r.L2 + g * GSZ, r.L2 + g * GSZ + N)
                pu = pup.tile([128, 2 * GSZ], F32, tag="pu")
                # both chunk0-reading matmuls first: the u-pipeline starts
                # as soon as chunk0's data lands, while chunk1 streams
                nc.tensor.matmul(pu[:, 0:N], wtb0[:, 0:128], r.ht[:, gs],
                                 start=True, stop=False)
                nc.tensor.matmul(pu[:, GSZ:GSZ + N], wtb0[:, 128:HP],
                                 r.ht[:, gs], start=True, stop=False)
                nc.tensor.matmul(pu[:, 0:N], wtb1[:, 0:128],
                                 r.ht[:, gs1], start=False, stop=True)
                nc.tensor.matmul(pu[:, GSZ:GSZ + N], wtb1[:, 128:HP],
                                 r.ht[:, gs1], start=False, stop=True)
                ut = utp.tile([128, 2 * GSZ], BF16, tag="ut")
                nc.scalar.activation(
                    ut[:].rearrange("p (k l) -> p k l", k=2)[:, :, 0:N],
                    pu[:].rearrange("p (k l) -> p k l", k=2)[:, :, 0:N],
                    AF.Tanh)
                r.ut[g] = ut

            def emit_scores(r, g):
                N = nsz(r, g)
                ut = r.ut.pop(g)
                p, j = divmod(g, 2)
                if j == 0:
                    sg = sgp.tile([128, 2 * GSZ], F32, tag="sg")
                    r.sg[p] = sg
                sg = r.sg[p]
                off = j * GSZ
                if r.pidx < 8:
                    s0 = uh0[:, 128 * r.pidx:128 * r.pidx + 128]
                    s1 = uh1[:, 128 * r.pidx:128 * r.pidx + 128]
                else:
                    s0 = usr0[:, 128 * r.i:128 * r.i + 128]
                    s1 = usr1[:, 128 * r.i:128 * r.i + 128]
                nc.tensor.matmul(sg[:, off:off + N], s0,
                                 ut[:, 0:N], start=True, stop=False)
                nc.tensor.matmul(sg[:, off:off + N], s1,
                                 ut[:, GSZ:GSZ + N],
                                 start=False, stop=True)

            def emit_exp(r, p):
                span = pair_span(r, p)
                sg = r.sg.pop(p)
                ab = abp.tile([128, 2 * GSZ], BF16, tag="ab")
                nc.scalar.activation(ab[:, 0:span], sg[:, 0:span], AF.Exp)
                r.ab[p] = ab

            def emit_v(r, p):
                span = pair_span(r, p)
                ab = r.ab.pop(p)
                l0 = 2 * p * GSZ
                for c in (0, 1):
                    col = vfin[:, 2 * r.i + c:2 * r.i + c + 1]
                    dm = dmp.tile([128, 1], BF16, tag="dm")
                    nc.vector.tensor_tensor_reduce(
                        dm[:].broadcast_to((128, span)),
                        r.ht[:, c * r.L2 + l0:c * r.L2 + l0 + span],
                        ab[:, 0:span],
                        scale=1.0,
                        scalar=(0.0 if p == 0 else col),
                        op0=ALU.mult,
                        op1=ALU.add,
                        accum_out=col)

            # ---- software-pipelined emission over (row, group) units ----
            # shortest row first: its h lands in <1us so the PE starts
            # ~12us earlier while the big rows stream in behind it
            slot_order = [BPC - 1] + list(range(BPC - 1))
            stream = []
            for i in slot_order:
                for g in range((16 * T_seq[i] + GSZ - 1) // GSZ):
                    stream.append((i, g))
            rows = {}
            for j, i in enumerate(slot_order):
                rows[i] = new_row(i, T_seq[i])
                rows[i].pidx = j
            # stream position -> (i, p) for the pair completing there
            pair_done = {}
            for k, (i, g) in enumerate(stream):
                r = rows[i]
                if g % 2 == 1 or g == r.G - 1:
                    pair_done[k] = (i, g // 2)
            SLAG, ELAG, VLAG = 1, 2, 3
            for k in range(len(stream) + VLAG):
                if 0 <= k < len(stream):
                    i, g = stream[k]
                    emit_u(rows[i], g)
                if 0 <= k - SLAG < len(stream):
                    i, g = stream[k - SLAG]
                    emit_scores(rows[i], g)
                if k - ELAG in pair_done:
                    i, p = pair_done[k - ELAG]
                    emit_exp(rows[i], p)
                if k - VLAG in pair_done:
                    i, p = pair_done[k - VLAG]
                    emit_v(rows[i], p)
            nc.gpsimd.dma_start(ov_d.ap()[:, :], vfin[:])

    return nc


_NC_CACHE = {}


def _get_nc(T_seq):
    if T_seq not in _NC_CACHE:
        _NC_CACHE[T_seq] = build(T_seq)
    return _NC_CACHE[T_seq]


def _prep_in_maps(short_perference, current_perference, W, bvec, length_input,
                  perm, T_seq):
    h = np.asarray(short_perference, dtype=np.float32)[0]      # [B, L, H]
    us = np.asarray(current_perference, dtype=np.float32)[0]   # [B, H]
    W = np.asarray(W, dtype=np.float32)
    bvec = np.asarray(bvec, dtype=np.float32)
    lens = np.asarray(length_input).astype(np.int64)

    wt = np.zeros((HP, HP), dtype=np.float32)                  # [c, o]
    wt[:H, :H] = W.T
    wt[HP - 1, :H] = bvec                                      # bias row
    wtb0 = wt[0:128].astype(BF16NP)
    wtb1 = wt[128:HP].astype(BF16NP)

    offs = []
    tot = 0
    for t in T_seq:
        offs.append(tot)
        tot += 2 * 16 * t
    in_maps = []
    for c in range(N_CORES):
        rows = perm[c]
        # host-side pre-transpose into the exact SBUF ht layout: per slot,
        # chunk0 [c 0:128, l] then chunk1 [c 128:256, l].  h rows at
        # l >= len are ZERO (incl. the ones/bias channel) so they
        # contribute exactly 0 to scores, numerator, and denominator —
        # the length mask costs nothing on device.
        hTc = np.zeros((128, tot), dtype=BF16NP)
        for i, r in enumerate(rows):
            n = int(lens[r])
            Lr = 16 * T_seq[i]
            tmp = np.zeros((Lr, HP), dtype=BF16NP)
            tmp[0:n, 0:H] = h[r, 0:n].astype(BF16NP)
            tmp[0:n, HP - 1] = BF16NP(1.0)
            o = offs[i]
            hTc[:, o:o + Lr] = tmp[:, 0:128].T
            hTc[:, o + Lr:o + 2 * Lr] = tmp[:, 128:HP].T
        usc = np.zeros((HP, BPC), dtype=np.float32)
        usc[0:H, :] = us[rows].T
        usr0 = np.repeat(usc[0:128].astype(BF16NP), 128, axis=1)
        usr1 = np.repeat(usc[128:HP].astype(BF16NP), 128, axis=1)
        head = [BPC - 1] + list(range(7))  # first 8 processed slots
        uh0 = np.concatenate([usr0[:, 128 * s:128 * s + 128] for s in head],
                             axis=1)
        uh1 = np.concatenate([usr1[:, 128 * s:128 * s + 128] for s in head],
                             axis=1)
        in_maps.append({
            "hT": np.ascontiguousarray(hTc),
            "wtb0": wtb0,
            "wtb1": wtb1,
            "usr0": np.ascontiguousarray(usr0),
            "usr1": np.ascontiguousarray(usr1),
            "uh0": np.ascontiguousarray(uh0),
            "uh1": np.ascontiguousarray(uh1),
        })
    return in_maps


def run(short_perference, current_perference, W, b, length_input,
        trace=False, **run_kwargs):
    lens = np.asarray(length_input).astype(np.int64)
    perm, T_seq = _schedule(lens)
    nc = _get_nc(T_seq)
    in_maps = _prep_in_maps(short_perference, current_perference, W, b,
                            lens, perm, T_seq)
    res = run_bass_kernel_spmd(nc, in_maps, list(range(N_CORES)),
                               trace=trace, **run_kwargs)
    v = np.zeros((B, H), dtype=np.float32)
    for c in range(N_CORES):
        ov = np.asarray(res.results[c]["ov"], dtype=np.float32)  # [128,2*BPC]
        for i, r in enumerate(perm[c]):
            denom = ov[127, 2 * i + 1]
            num = np.concatenate([ov[:, 2 * i], ov[0:H1, 2 * i + 1]])
            v[r] = num / denom
    return v, res


def kernel(short_perference, current_perference, W, b, current_batch,
           length_input):
    v, _ = run(short_perference, current_perference, W, b, length_input)
    return v.astype(np.float32)
